# revision 31
# baseline (speedup 1.0000x reference)
"""Trainium2 Bass kernel for nn_Attention_90752658965090.

Computes, per batch element b of x[16, 512, 64, 64] (n = h*w = 4096):
  xn   = GroupNorm(8 groups, eps=1e-5, affine)(x[b])            # [512, 4096]
  q,k,v = split(qkv_w @ xn + qkv_b)                             # each [512, 4096]
  attn = softmax((q @ k^T) * 512**-0.5, axis=-1)                # [512, 512]
  out  = proj_w @ (attn @ v) + proj_b                           # [512, 4096]
  y[b] = x[b] + out

Algebraic restructuring (cuts PE work ~2x vs the direct pipeline):
GroupNorm is per-channel affine xn = scl*x + bia, so with
  Aq = Wq diag(scl),  cq = Wq bia + bq   (same for k, v)
  Gx = x x^T,  sx = x @ 1                (Gram matrix of RAW x)
  logits = Aq Gx Ak^T + cq u^T + w ck^T, u = Ak sx + n ck, w = Aq sx
i.e. q/k are never materialized; the n=4096 contraction happens once in
Gx (which also serves the stats via bn_stats) instead of three times
(q, k, q@k^T). After softmax e/z, fold proj into a [512,512] matrix:
  N' = proj_w diag(1/z) e
  y  = x + (N' Av) x + (N' cv + proj_b) 1^T
so attn@v and proj over n collapse into one [512,512] @ [512,4096].

Sharding: data-parallel over batch, 2 batch elements per core on 8 cores.
All big matmuls are float32r with 512-wide moving operands (full PE rate).
"""
import sys

sys.path.insert(0, "/opt/trn_rl_repo")

import numpy as np

import concourse.bass as bass
import concourse.mybir as mybir
import concourse.tile as tile
from concourse import bacc

B, C, HW = 16, 512, 4096
NCORES = 8
BPC = B // NCORES          # batches per core
P = 128
CT = C // P                # 4 c-tiles
NCH = HW // 512            # 8 n-chunks of 512
GROUPS = 8
EPS = 1e-5
INV_N = 1.0 / (C // GROUPS)   # bn_aggr normalizes over HW; combine 64 ch/group
SCALE = float(C) ** -0.5

F32 = mybir.dt.float32
F32R = mybir.dt.float32r
AX = mybir.AxisListType
OP = mybir.AluOpType
AF = mybir.ActivationFunctionType


def build_program(repeat=1):
    nc = bacc.Bacc("TRN2", target_bir_lowering=False, debug=False, num_devices=NCORES)

    x_d = nc.dram_tensor("x", [BPC, C, HW], F32R, kind="ExternalInput")
    y_d = nc.dram_tensor("y", [BPC, C, HW], F32, kind="ExternalOutput")
    wqkvT_d = nc.dram_tensor("wqkvT", [C, 3 * C], F32R, kind="ExternalInput")
    wvr_d = nc.dram_tensor("wvr", [C, C], F32R, kind="ExternalInput")
    wprojT_d = nc.dram_tensor("wprojT", [C, C], F32R, kind="ExternalInput")
    qkb2_d = nc.dram_tensor("qkb2", [1, 2 * C], F32, kind="ExternalInput")
    cols_d = nc.dram_tensor("cols", [P, 4 * CT], F32, kind="ExternalInput")
    indp_d = nc.dram_tensor("indp", [P, GROUPS * CT], F32, kind="ExternalInput")
    indT_d = nc.dram_tensor("indT", [GROUPS, C], F32, kind="ExternalInput")
    ident_d = nc.dram_tensor("ident", [P, P], F32R, kind="ExternalInput")

    from contextlib import ExitStack, nullcontext
    with tile.TileContext(nc) as tc, ExitStack() as ctx:
        wgt = ctx.enter_context(tc.tile_pool(name="wgt", bufs=1))
        xxp = ctx.enter_context(tc.tile_pool(name="xxp", bufs=40))
        xtp = ctx.enter_context(tc.tile_pool(name="xtp", bufs=3))
        bbp = ctx.enter_context(tc.tile_pool(name="bbp", bufs=4))   # per-batch [P,512]s
        nwp = ctx.enter_context(tc.tile_pool(name="nwp", bufs=8))   # cross-batch alive
        yop = ctx.enter_context(tc.tile_pool(name="yop", bufs=6))
        sm = ctx.enter_context(tc.tile_pool(name="sm", bufs=2))
        psG = ctx.enter_context(tc.tile_pool(name="psG", bufs=4, space=bass.MemorySpace.PSUM))
        psB = ctx.enter_context(tc.tile_pool(name="psB", bufs=4, space=bass.MemorySpace.PSUM))

        # --- constants / weights ---
        cols = wgt.tile([P, 4 * CT], F32, tag="cols")
        nc.sync.dma_start(cols[:], cols_d[:])
        indp = wgt.tile([P, GROUPS * CT], F32, tag="indp")
        nc.sync.dma_start(indp[:], indp_d[:])
        indT8 = wgt.tile([GROUPS, C], F32, tag="indT8")
        nc.sync.dma_start(indT8[:], indT_d[:])
        ident = wgt.tile([P, P], F32R, tag="ident")
        nc.sync.dma_start(ident[:], ident_d[:])
        qkb2 = wgt.tile([1, 2 * C], F32, tag="qkb2")
        nc.sync.dma_start(qkb2[:], qkb2_d[:])
        wqkvT = []
        for t in range(CT):
            w1 = wgt.tile([P, 3 * C], F32R, tag=f"wqkv{t}")
            nc.sync.dma_start(w1[:], wqkvT_d[t * P:(t + 1) * P, :])
            wqkvT.append(w1)
        wvr = []
        for t in range(CT):
            w2 = wgt.tile([P, C], F32R, tag=f"wvr{t}")
            nc.sync.dma_start(w2[:], wvr_d[t * P:(t + 1) * P, :])
            wvr.append(w2)
        wprojT = []
        for t in range(CT):
            w3 = wgt.tile([P, C], F32R, tag=f"wproj{t}")
            nc.sync.dma_start(w3[:], wprojT_d[t * P:(t + 1) * P, :])
            wprojT.append(w3)
        eps_t = wgt.tile([GROUPS, 1], F32, tag="eps")
        nc.vector.memset(eps_t[:], EPS)
        nwc = [cols[:, 4 * t + 0:4 * t + 1] for t in range(CT)]
        nbc = [cols[:, 4 * t + 1:4 * t + 2] for t in range(CT)]
        vbc = [cols[:, 4 * t + 2:4 * t + 3] for t in range(CT)]
        pbc = [cols[:, 4 * t + 3:4 * t + 4] for t in range(CT)]
        indt = [indp[:, GROUPS * t:GROUPS * (t + 1)] for t in range(CT)]
        indTt = [indT8[:, t * P:(t + 1) * P] for t in range(CT)]

        # ---- phase A: stream x (resident), bn stats, transpose, Gram accum ----
        # Software-pipelined: the Gram matmuls for block k are emitted after
        # the transposes of block k+1, so the in-order PE queue never stalls
        # on the ACT psum->sbuf copy of xT.
        def emit_gx(gx, pend):
            # Gx is symmetric: row-tiles 2,3 only accumulate columns 256:512
            # (their left half is reconstructed by transposing the upper-right
            # quadrant at phase-B start). Both moving widths stay >=256, so
            # every matmul runs at full f32r rate.
            xT, blk = pend
            for it in range(CT):
                cl = 0 if it < 2 else 256
                nc.tensor.matmul(
                    gx[it][:, cl:512], xT[:, it * P:(it + 1) * P], xT[:, cl:512],
                    start=(blk == 0), stop=(blk == NCH * 4 - 1),
                    skip_group_check=True)

        def phase_a_chunk(b, ch, xx, st6, gx, pend):
            for t in range(CT):
                xt = xxp.tile([P, 512], F32R, tag="xx", name=f"x{b}_{ch}_{t}")
                nc.sync.dma_start(
                    xt[:], x_d[b, t * P:(t + 1) * P, ch * 512:(ch + 1) * 512])
                xx[ch][t] = xt
                nc.vector.bn_stats(st6[t][:, ch, :], xt[:])
            for ns in range(4):
                tp = psB.tile([P, 512], F32R, tag="wk", name=f"tp{b}_{ch}_{ns}")
                for t in range(CT):
                    nc.tensor.transpose(
                        tp[:, t * P:(t + 1) * P],
                        xx[ch][t][:, ns * P:(ns + 1) * P], ident[:])
                xT = xtp.tile([P, 512], F32R, tag="xT", name=f"xT{b}_{ch}_{ns}")
                nc.scalar.activation(xT[:], tp[:], AF.Copy, scale=1.0)
                if pend is not None:
                    emit_gx(gx, pend)
                pend = (xT, ch * 4 + ns)
            return pend

        # ---- phase A2: finalize stats -> scl/bia/sx and batch smalls ----
        def stats_final(b, st6):
            ms_tiles = []
            for t in range(CT):
                mv = sm.tile([P, 2], F32, tag="mv", name=f"mv{b}_{t}", bufs=4)
                nc.vector.bn_aggr(mv[:], st6[t][:])
                ms = sm.tile([P, 2], F32, tag="ms", name=f"ms{b}_{t}", bufs=4)
                nc.vector.tensor_copy(ms[:, 0:1], mv[:, 0:1])
                nc.vector.scalar_tensor_tensor(
                    out=ms[:, 1:2], in0=mv[:, 0:1], scalar=mv[:, 0:1],
                    in1=mv[:, 1:2], op0=OP.mult, op1=OP.add)
                ms_tiles.append(ms)
            gps = psB.tile([GROUPS, 2], F32, tag="wk", name=f"gps{b}")
            for t in range(CT):
                nc.tensor.matmul(gps[:], indt[t], ms_tiles[t][:],
                                 start=(t == 0), stop=(t == CT - 1))
            gsb = sm.tile([GROUPS, 2], F32, tag="gsb", name=f"gsb{b}")
            nc.scalar.activation(gsb[:], gps[:], AF.Copy, scale=INV_N)
            m2 = sm.tile([GROUPS, 1], F32, tag="m2", name=f"m2_{b}")
            nc.vector.tensor_tensor(m2[:], gsb[:, 0:1], gsb[:, 0:1], op=OP.mult)
            var = sm.tile([GROUPS, 1], F32, tag="var", name=f"var{b}")
            nc.vector.tensor_tensor(var[:], gsb[:, 1:2], m2[:], op=OP.subtract)
            sq = sm.tile([GROUPS, 1], F32, tag="sq", name=f"sq{b}")
            nc.scalar.activation(sq[:], var[:], AF.Sqrt, bias=eps_t[:])
            mrs = sm.tile([GROUPS, 2], F32, tag="mrs", name=f"mrs{b}")
            nc.vector.tensor_copy(mrs[:, 0:1], gsb[:, 0:1])
            nc.vector.reciprocal(mrs[:, 1:2], sq[:])
            scl, bia, biascl, sx = [], [], [], []
            for t in range(CT):
                bps = psB.tile([P, 2], F32, tag="wk", name=f"bps{b}_{t}")
                nc.tensor.matmul(bps[:], indTt[t], mrs[:], start=True, stop=True)
                s_ = sm.tile([P, 1], F32, tag="scl", name=f"scl{b}_{t}", bufs=8)
                nc.vector.tensor_tensor(s_[:], bps[:, 1:2], nwc[t], op=OP.mult)
                tmpb = sm.tile([P, 1], F32, tag="tmpb", name=f"tmpb{b}_{t}")
                nc.vector.tensor_tensor(tmpb[:], bps[:, 0:1], s_[:], op=OP.mult)
                b_ = sm.tile([P, 1], F32R, tag="bia", name=f"bia{b}_{t}", bufs=8)
                nc.vector.tensor_tensor(b_[:], nbc[t], tmpb[:], op=OP.subtract)
                inv_s = sm.tile([P, 1], F32, tag="invs", name=f"invs{b}_{t}")
                nc.vector.reciprocal(inv_s[:], s_[:])
                bc = sm.tile([P, 1], F32R, tag="biascl", name=f"biascl{b}_{t}", bufs=8)
                nc.vector.tensor_tensor(bc[:], b_[:], inv_s[:], op=OP.mult)
                # sx = n * mean (per channel); stationary for the rank-2 rows
                sx_ = sm.tile([P, 1], F32R, tag="sx", name=f"sx{b}_{t}", bufs=8)
                nc.vector.tensor_scalar_mul(sx_[:], ms_tiles[t][:, 0:1], float(HW))
                scl.append(s_)
                bia.append(b_)
                biascl.append(bc)
                sx.append(sx_)
            return scl, bia, biascl, sx

        def phase_a2(b, scl, bia, biascl, sx):
            # scaled weight copies: wqS = diag(scl) Wq^T (i.e. Aq^T), wkS = Ak^T
            wqS, wkS = [], []
            for t in range(CT):
                wq_ = bbp.tile([P, 512], F32R, tag="wqS", name=f"wqS{b}_{t}")
                nc.scalar.activation(wq_[:], wqkvT[t][:, 0:512], AF.Copy,
                                     scale=scl[t][:])
                wqS.append(wq_)
            for t in range(CT):
                wk_ = bbp.tile([P, 512], F32R, tag="wkS", name=f"wkS{b}_{t}")
                nc.scalar.activation(wk_[:], wqkvT[t][:, 512:1024], AF.Copy,
                                     scale=scl[t][:])
                wkS.append(wk_)
            # rank-2 rows via M=1 matmuls: [1,512] psum rows
            # row0 = Wq bia (cq pre-bias), row1 = Wk bia (ck pre-bias),
            # row2 = Aq sx (w), row3 = Ak sx (u pre-bias)
            rows = [psB.tile([1, 512], F32, tag="wk", name=f"rows{b}_{i}")
                    for i in range(4)]
            for ct in range(CT):
                st = (ct == 0)
                sp = (ct == CT - 1)
                nc.tensor.matmul(rows[0][:], biascl[ct][:], wqS[ct][:],
                                 start=st, stop=sp, skip_group_check=True)
                nc.tensor.matmul(rows[1][:], biascl[ct][:], wkS[ct][:],
                                 start=st, stop=sp, skip_group_check=True)
                nc.tensor.matmul(rows[2][:], sx[ct][:], wqS[ct][:],
                                 start=st, stop=sp, skip_group_check=True)
                nc.tensor.matmul(rows[3][:], sx[ct][:], wkS[ct][:],
                                 start=st, stop=sp, skip_group_check=True)
            cqr = sm.tile([1, 512], F32R, tag="cqr", name=f"cqr{b}", bufs=1)
            wr = sm.tile([1, 512], F32R, tag="wr", name=f"wr{b}", bufs=1)
            ckr = sm.tile([1, 512], F32R, tag="ckr", name=f"ckr{b}", bufs=1)
            ur = sm.tile([1, 512], F32R, tag="ur", name=f"ur{b}", bufs=1)
            nc.vector.tensor_tensor(cqr[:], rows[0][:], qkb2[0:1, 0:512],
                                    op=OP.add)              # cq = Wq bia + bq
            nc.vector.tensor_copy(wr[:], rows[2][:])        # w = Aq sx
            nc.vector.tensor_tensor(ckr[:], rows[1][:], qkb2[0:1, 512:1024],
                                    op=OP.add)              # ck = Wk bia + bk
            nc.vector.scalar_tensor_tensor(
                out=ur[:], in0=ckr[:], scalar=float(HW),
                in1=rows[3][:], op0=OP.mult, op1=OP.add)    # u = Ak sx + n ck
            b2 = (cqr, wr)
            mv2 = (ur, ckr)
            # cv = Wv bia + bv. Row matmul (N=1-moving f32r matmuls fail the
            # ISA check), then DMA-scatter the row into column layout.
            cvr = psB.tile([1, 512], F32, tag="wk", name=f"cvr{b}")
            for ct in range(CT):
                nc.tensor.matmul(cvr[:], bia[ct][:],
                                 wqkvT[ct][:, 2 * C:3 * C],
                                 start=(ct == 0), stop=(ct == CT - 1))
            cvrs = sm.tile([1, 512], F32R, tag="rowtmp", name=f"cvrs{b}", bufs=1)
            nc.vector.tensor_copy(cvrs[:], cvr[:])
            cvS = sm.tile([P, CT], F32R, tag="cvS", name=f"cvS{b}", bufs=1)
            for dt in range(CT):
                nc.sync.dma_start(cvS[:, dt:dt + 1],
                                  cvrs[0:1, dt * P:(dt + 1) * P])
            cv4 = sm.tile([P, CT], F32R, tag="cv4", name=f"cv4_{b}")
            for dt in range(CT):
                nc.vector.tensor_tensor(cv4[:, dt:dt + 1], cvS[:, dt:dt + 1],
                                        vbc[dt], op=OP.add)
            return wqS, wkS, b2, mv2, cv4

        # ---- phase B: Gram -> logits -> softmax -> fused proj matrices ----
        def phase_b(b, gx, wqS, wkS, b2, mv2, cv4, scl, emit_mids=()):
            gxs = []
            for it in range(CT):
                g_ = bbp.tile([P, 512], F32R, tag="gel", name=f"gxs{b}_{it}")
                cl = 0 if it < 2 else 256
                nc.scalar.activation(g_[:, cl:512], gx[it][:, cl:512], AF.Copy,
                                     scale=1.0)
                gxs.append(g_)
            if len(emit_mids) > 0:
                emit_mids[0]()  # C(b-1) chunk: PE work overlapping gxs copies
            # reconstruct the lower-left quadrant of Gx by symmetry:
            # gxs[2|3][:, 0:256] = transpose of gxs[0|1][:, 256:512]
            tq = psB.tile([P, 512], F32R, tag="wk", name=f"tq{b}")
            nc.tensor.transpose(tq[:, 0:P], gxs[0][:, 256:384], ident[:])
            nc.tensor.transpose(tq[:, P:2 * P], gxs[1][:, 256:384], ident[:])
            nc.tensor.transpose(tq[:, 2 * P:3 * P], gxs[0][:, 384:512], ident[:])
            nc.tensor.transpose(tq[:, 3 * P:4 * P], gxs[1][:, 384:512], ident[:])
            nc.scalar.activation(gxs[2][:, 0:256], tq[:, 0:256], AF.Copy,
                                 scale=1.0)
            nc.scalar.activation(gxs[3][:, 0:256], tq[:, 256:512], AF.Copy,
                                 scale=1.0)
            # T = Gx @ Ak^T   [512 i, 512 o]
            tgs = []
            for it in range(CT):
                tT = psB.tile([P, 512], F32, tag="wk", name=f"tT{b}_{it}")
                for jt in range(CT):
                    nc.tensor.matmul(tT[:], gxs[jt][:, it * P:(it + 1) * P],
                                     wkS[jt][:],
                                     start=(jt == 0), stop=(jt == CT - 1))
                tg = bbp.tile([P, 512], F32R, tag="tgw", name=f"tgs{b}_{it}")
                nc.scalar.activation(tg[:], tT[:], AF.Copy, scale=1.0)
                tgs.append(tg)
            if len(emit_mids) > 1:
                emit_mids[1]()  # C(b-1) final chunk: overlaps tgs copies
            # logits = rank2 + Aq T ; then softmax (scaled exp straight off psum)
            ee, rz = [], []
            cqr, wr = b2
            ur, ckr = mv2
            for mt in range(CT):
                lg = psB.tile([P, 512], F32, tag="wk", name=f"lg{b}_{mt}")
                nc.tensor.matmul(lg[:], cqr[:, mt * P:(mt + 1) * P], ur[:],
                                 start=True, stop=False, skip_group_check=True)
                nc.tensor.matmul(lg[:], wr[:, mt * P:(mt + 1) * P], ckr[:],
                                 start=False, stop=False, skip_group_check=True)
                for it in range(CT):
                    nc.tensor.matmul(lg[:], wqS[it][:, mt * P:(mt + 1) * P],
                                     tgs[it][:],
                                     start=False, stop=(it == CT - 1),
                                     skip_group_check=True)
                nmxr = sm.tile([P, 1], F32, tag="nmxr", name=f"nmxr{b}_{mt}", bufs=4)
                nc.vector.reduce_max(nmxr[:], lg[:], axis=AX.X, negate=True)
                nmx = sm.tile([P, 1], F32, tag="nmx", name=f"nmx{b}_{mt}", bufs=4)
                nc.vector.tensor_scalar_mul(nmx[:], nmxr[:], SCALE)
                e_ = bbp.tile([P, 512], F32R, tag="gel", name=f"e{b}_{mt}")
                z_ = sm.tile([P, 1], F32, tag="z", name=f"z{b}_{mt}", bufs=4)
                nc.scalar.activation(e_[:], lg[:], AF.Exp,
                                     bias=nmx[:], scale=SCALE, accum_out=z_[:])
                r_ = sm.tile([P, 1], F32, tag="r", name=f"r{b}_{mt}", bufs=4)
                nc.vector.reciprocal(r_[:], z_[:])
                # attn row-normalization folded into e in place: e <- e/z
                nc.vector.tensor_scalar_mul(e_[:], e_[:], r_[:])
                ee.append(e_)
                rz.append(r_)
            # N'^T = (diag(1/z) e)^T proj_w^T   [512 d, 512 o]
            nts = []
            for dt in range(CT):
                nt = psB.tile([P, 512], F32, tag="wk", name=f"nt{b}_{dt}")
                for ct in range(CT):
                    nc.tensor.matmul(nt[:], ee[ct][:, dt * P:(dt + 1) * P],
                                     wprojT[ct][:],
                                     start=(ct == 0), stop=(ct == CT - 1))
                n_ = bbp.tile([P, 512], F32R, tag="tgw", name=f"nts{b}_{dt}")
                nc.scalar.activation(n_[:], nt[:], AF.Copy, scale=1.0)
                nts.append(n_)
            # (N' Av)^T = diag(scl) Wv^T N'^T   [512 c, 512 o], scl folded in copy
            nws = []
            for ct in range(CT):
                nw_ = psB.tile([P, 512], F32, tag="wk", name=f"nw{b}_{ct}")
                for dt in range(CT):
                    nc.tensor.matmul(nw_[:], wvr[dt][:, ct * P:(ct + 1) * P],
                                     nts[dt][:],
                                     start=(dt == 0), stop=(dt == CT - 1))
                nn_ = nwp.tile([P, 512], F32R, tag="nws", name=f"nws{b}_{ct}")
                nc.scalar.activation(nn_[:], nw_[:], AF.Copy, scale=scl[ct][:])
                nws.append(nn_)
            # q = N' cv + proj_b: row matmul then DMA-scatter to columns
            qrp = psB.tile([1, 512], F32, tag="wk", name=f"qrp{b}")
            for dt in range(CT):
                nc.tensor.matmul(qrp[:], cv4[:, dt:dt + 1], nts[dt][:],
                                 start=(dt == 0), stop=(dt == CT - 1))
            qrs = sm.tile([1, 512], F32R, tag="rowtmp", name=f"qrs{b}", bufs=1)
            nc.vector.tensor_copy(qrs[:], qrp[:])
            qcS = nwp.tile([P, CT], F32R, tag="qcS", name=f"qcS{b}")
            for ot in range(CT):
                nc.sync.dma_start(qcS[:, ot:ot + 1],
                                  qrs[0:1, ot * P:(ot + 1) * P])
            qc4 = nwp.tile([P, CT], F32, tag="qc4", name=f"qc4_{b}")
            for ot in range(CT):
                nc.vector.tensor_tensor(qc4[:, ot:ot + 1], qcS[:, ot:ot + 1],
                                        pbc[ot], op=OP.add)
            return nws, qc4

        # ---- phase C: y = x + (N'Av) x + q_col ----
        def phase_c_chunk(b, ch, xx, nws, qc4):
            for ot in range(CT):
                po = psB.tile([P, 512], F32, tag="wk", name=f"po{b}_{ch}_{ot}")
                for ct in range(CT):
                    nc.tensor.matmul(po[:], nws[ct][:, ot * P:(ot + 1) * P],
                                     xx[ch][ct][:],
                                     start=(ct == 0), stop=(ct == CT - 1))
                yt = yop.tile([P, 512], F32, tag="y", name=f"yt{b}_{ch}_{ot}")
                nc.vector.scalar_tensor_tensor(
                    out=yt[:], in0=po[:], scalar=qc4[:, ot:ot + 1],
                    in1=xx[ch][ot][:], op0=OP.add, op1=OP.add)
                nc.sync.dma_start(
                    y_d[b, ot * P:(ot + 1) * P, ch * 512:(ch + 1) * 512], yt[:])

        rep_cm = tc.For_i(0, repeat, 1) if repeat > 1 else nullcontext()
        with rep_cm:
            state = {}
            for b in range(BPC):
                xx = [[None] * CT for _ in range(NCH)]
                st6 = [sm.tile([P, NCH, 6], F32, tag="st6", name=f"st6_{b}_{t}")
                       for t in range(CT)]
                gx = [psG.tile([P, 512], F32, tag="gx", name=f"gx{b}_{it}")
                      for it in range(CT)]
                prev = state.get(b - 1)
                # A(b) chunks, interleaved with C(b-1) chunks 0..5 at lag 2.
                # C(b-1, ch-2) MUST be emitted before A(b, ch): the A-phase
                # DMA reuses the xx ring slot whose last reader is that C
                # chunk's PE/DVE work; emitting C after would deadlock the
                # in-order PE queue.
                pend = None
                for ch in range(NCH):
                    if prev is not None and ch >= 2:
                        phase_c_chunk(b - 1, ch - 2, *prev)
                    pend = phase_a_chunk(b, ch, xx, st6, gx, pend)
                emit_gx(gx, pend)
                if prev is not None:
                    phase_c_chunk(b - 1, NCH - 2, *prev)  # C6: covers stats DVE
                scl, bia, biascl, sx = stats_final(b, st6)
                wqS, wkS, b2, mv2, cv4 = phase_a2(b, scl, bia, biascl, sx)
                emit_mids = ()
                if prev is not None:
                    emit_mids = (
                        lambda pb=b - 1, pv=prev: phase_c_chunk(pb, NCH - 1, *pv),
                    )
                nws, qc4 = phase_b(b, gx, wqS, wkS, b2, mv2, cv4, scl,
                                   emit_mids=emit_mids)
                state[b] = (xx, nws, qc4)
                state.pop(b - 1, None)
            # drain last batch
            xx, nws, qc4 = state[BPC - 1]
            for ch in range(NCH):
                phase_c_chunk(BPC - 1, ch, xx, nws, qc4)

    nc.compile()
    return nc


_NC = None


def _get_program():
    global _NC
    if _NC is None:
        _NC = build_program()
    return _NC


def make_in_maps(x, norm_w, norm_b, qkv_w, qkv_b, proj_w, proj_b):
    x = np.asarray(x, dtype=np.float32).reshape(B, C, HW)
    qkv_w = np.asarray(qkv_w, dtype=np.float32)
    proj_w = np.asarray(proj_w, dtype=np.float32)
    qkv_b = np.asarray(qkv_b, dtype=np.float32)
    nw = np.asarray(norm_w, np.float32).reshape(CT, P)
    nb = np.asarray(norm_b, np.float32).reshape(CT, P)
    vb = qkv_b[2 * C:].reshape(CT, P)
    pb = np.asarray(proj_b, np.float32).reshape(CT, P)
    cols = np.empty((P, 4 * CT), np.float32)
    for t in range(CT):
        cols[:, 4 * t + 0] = nw[t]
        cols[:, 4 * t + 1] = nb[t]
        cols[:, 4 * t + 2] = vb[t]
        cols[:, 4 * t + 3] = pb[t]
    ind = np.eye(GROUPS, dtype=np.float32)[np.arange(C) // (C // GROUPS)]  # [C, G]
    indp = np.empty((P, GROUPS * CT), np.float32)
    for t in range(CT):
        indp[:, GROUPS * t:GROUPS * (t + 1)] = ind[t * P:(t + 1) * P]
    common = {
        "wqkvT": np.ascontiguousarray(qkv_w.T),
        "wvr": np.ascontiguousarray(qkv_w[2 * C:, :]),
        "wprojT": np.ascontiguousarray(proj_w.T),
        "qkb2": np.ascontiguousarray(qkv_b[:2 * C].reshape(1, 2 * C)),
        "cols": cols,
        "indp": indp,
        "indT": np.ascontiguousarray(ind.T),
        "ident": np.eye(P, dtype=np.float32),
    }
    return [
        {"x": np.ascontiguousarray(x[i * BPC:(i + 1) * BPC]), **common}
        for i in range(NCORES)
    ]


def _wait_device(max_wait=600):
    """The axon-tunneled device can be transiently unrecoverable right after
    another process's teardown; poll with a tiny op until it responds."""
    import time
    import jax
    import jax.numpy as jnp
    t0 = time.time()
    while True:
        try:
            v = float((jnp.ones((4, 4)) @ jnp.ones((4, 4))).sum())
            assert v == 64.0
            return
        except Exception:
            if time.time() - t0 > max_wait:
                raise
            time.sleep(30)


def run(inputs, trace=False):
    import time
    from concourse.bass_utils import run_bass_kernel_spmd
    nc = _get_program()
    in_maps = make_in_maps(**inputs)
    last_err = None
    for attempt in range(3):
        try:
            if attempt > 0:
                time.sleep(60)
            _wait_device()
            r = run_bass_kernel_spmd(nc, in_maps, list(range(NCORES)), trace=trace)
            break
        except Exception as e:
            last_err = e
    else:
        raise last_err
    y = np.concatenate([r.results[i]["y"] for i in range(NCORES)], axis=0)
    return y.reshape(B, C, 64, 64), r


def kernel(**inputs):
    y, _ = run(inputs, trace=False)
    return y
